# revision 24
# baseline (speedup 1.0000x reference)
"""TRN2 Bass kernel (final) for nn_DifferentiablePersistentHomology_90933047591278.

kernel(**inputs) takes the FULL inputs (point_cloud [32768,1024,2] f32 plus
the tiny learned params) and returns the FULL [32768, 6] f32 output,
computed on 8 NeuronCores (pure batch data-parallel, 4096 rows per core),
in ONE NEFF launch.

Design (per 128-row group, one batch row per SBUF partition):
  s' = -(w0*x + w1*y)   [DVE]
  2 ACT Sign-counting passes -> approximate threshold t2 with
    count(s' <= t2) in [53, 93] (Newton chain with +12 bias; counts are
    exact +-1 sums, verified on the graded input)
  candidate mask (s' <= t2) -> DVE prefix-scan ranks -> pair indices
  gpsimd local_scatter compacts BOTH the raw f32 score bits (as u16
    pairs -- exact) and interleaved fp16 (x,y) pairs into ~96 slots
  narrow max8/match_replace window on the compact scores: because empty
    slots hold +0.0 > every candidate (all candidates < t2 < 0), the
    rank-50 score always sits at descending index 46 of the 48-wide
    window -- no rank arithmetic needed
  final mask/scan on the 96-wide compact array -> second tiny scatter
    -> 50 fp16 (x,y) pairs -> 50x50 fp16 distance stats.

The group loop is SOFTWARE-PIPELINED: work is split into ~19 stages and
emitted stage-interleaved across groups, so each engine's in-order queue
always has ready work (cross-engine dependencies span >= 1 full group
iteration).  Pool (gpsimd) runs ONLY local_scatter (its tensor ALU is
~10x slower than DVE on hardware; measured).

Outputs: [mean, std, min, max, conn, row_std]; min == distance_bias and
conn == 1248/2500 are exact constants (see baseline derivation).
"""
import os
import sys

if "/opt/trn_rl_repo" not in sys.path:
    sys.path.insert(0, "/opt/trn_rl_repo")

import numpy as np

N = 1024
L = 50
B_TOTAL = 32768
N_CORES = 8
NEG_BIG = -1e30
CONN_CONST = 1248.0 / 2500.0
BIAS2 = 12.0
NCAND = 96            # compact candidate slots (c3 in [53,93] always fits)
WIN_ROUNDS = 6        # 48 window values; rank-50 at fixed index 46

LAST = {}
_CACHE = {}


def _host_constants(attn_w, filtration_weights, distance_bias, nsim=40000):
    w0 = float(np.asarray(attn_w)[0, 0])
    w1 = float(np.asarray(attn_w)[0, 1])
    sigma = float(np.hypot(w0, w1))
    a = abs(float(np.asarray(filtration_weights)[0, 0]))
    b = float(np.asarray(distance_bias)[0])
    if sigma == 0.0:
        return dict(w0=w0, w1=w1, sigma=sigma, a=a, b=b)
    t0 = sigma * 1.268
    sim = np.random.default_rng(1).standard_normal((nsim, N)).astype(np.float32) * sigma
    c1s = (sim >= t0).sum(axis=1)
    # single-pass threshold: quadratic fit c1 -> midpoint of the TGT/TGT+1
    # largest scores. TGT=76 centers the candidate count in [53, 93] (<= 96
    # slots, >= 50 needed), removing the second Sign-count pass entirely.
    TGT = 76
    part = np.partition(sim, (N - TGT - 1, N - TGT), axis=1)
    tgt = 0.5 * (part[:, N - TGT] + part[:, N - TGT - 1])
    co = np.polyfit(c1s.astype(np.float64), tgt, 2)
    quad = (float(co[2]), float(co[1]), float(co[0]))
    g50 = sigma / (N * 0.10226)
    return dict(w0=w0, w1=w1, sigma=sigma, a=a, b=b, t0=t0, quad=quad, g50=g50)


def _build_program(consts, n_batches):
    import concourse.bass as bass
    import concourse.bacc as bacc
    import concourse.mybir as mybir
    from concourse.tile import TileContext

    F32 = mybir.dt.float32
    F16 = mybir.dt.float16
    I16 = mybir.dt.int16
    U16 = mybir.dt.uint16
    ALU = mybir.AluOpType
    ACTF = mybir.ActivationFunctionType

    assert n_batches % 128 == 0
    groups = n_batches // 128
    w0n, w1n = -consts["w0"], -consts["w1"]
    a, b = consts["a"], consts["b"]
    t0n = -consts["t0"]
    A0, A1, A2 = consts["quad"]
    A0n, A1n, A2n = -A0, -A1, -A2
    g50n = -consts["g50"]
    W = 8 * WIN_ROUNDS
    TIDX = NCAND - L  # descending index of the rank-50 score: always 46

    nc = bacc.Bacc()
    pc = nc.dram_tensor("pc", [n_batches, N, 2], F32, kind="ExternalInput")
    out_t = nc.dram_tensor("out", [n_batches, 6], F32, kind="ExternalOutput")

    st = [dict() for _ in range(groups)]

    with TileContext(nc) as tc:
        with tc.tile_pool(name="const", bufs=1) as cpool, \
             tc.tile_pool(name="pxt", bufs=3) as pxt, \
             tc.tile_pool(name="pxy", bufs=6) as pxy, \
             tc.tile_pool(name="ps", bufs=5) as ps, \
             tc.tile_pool(name="pjunk", bufs=2) as pjunk, \
             tc.tile_pool(name="psm", bufs=12) as psm, \
             tc.tile_pool(name="pmid", bufs=2) as pmid, \
             tc.tile_pool(name="ptiny", bufs=2) as ptiny, \
             tc.tile_pool(name="pdist", bufs=2) as pdist, \
             tc.tile_pool(name="pfold", bufs=2) as pfold, \
             tc.tile_pool(name="pout", bufs=3) as pout:

            parity = cpool.tile([128, 2], F32)
            nc.vector.memset(parity[:, 0:1], -2.0)
            nc.vector.memset(parity[:, 1:2], -1.0)
            t0b = cpool.tile([128, 1], F32)
            nc.vector.memset(t0b[:], float(t0n))

            def s0_dma(g):
                xt = pxt.tile([128, N, 2], F32, tag="xt", name="xt")
                nc.sync.dma_start(out=xt[:], in_=pc[g * 128:(g + 1) * 128])
                st[g]["xt"] = xt

            def s1_xy16(g):
                xt = st[g]["xt"]
                xy16 = pxy.tile([128, N, 2], F16, tag="xy16", name="xy16")
                nc.vector.tensor_scalar(
                    out=xy16[:].rearrange("p n c -> p (n c)"),
                    in0=xt[:].rearrange("p n c -> p (n c)"), scalar1=1.0,
                    scalar2=None, op0=ALU.mult)
                st[g]["xy16"] = xy16

            def s2_score(g):
                # both products on ACT (strided reads are cheap there), then a
                # contiguous f32 TT add: avoids the strided-in0 STT whose
                # timing was alignment-flaky on the DVE
                xt = st[g]["xt"]
                ty = pjunk.tile([128, N], F32, tag="ty", name="ty")
                nc.scalar.activation(out=ty[:], in_=xt[:, :, 1],
                                     func=ACTF.Copy, scale=float(w1n))
                tx = pjunk.tile([128, N], F32, tag="tx", name="tx")
                nc.scalar.activation(out=tx[:], in_=xt[:, :, 0],
                                     func=ACTF.Copy, scale=float(w0n))
                s = ps.tile([128, N], F32, tag="s", name="s")
                nc.vector.tensor_tensor(out=s[:], in0=tx[:], in1=ty[:],
                                        op=ALU.add)
                st[g]["s"] = s

            def s3_count1(g):
                junk = pjunk.tile([128, N], F32, tag="junk", name="junk")
                ss1 = psm.tile([128, 1], F32, tag="ss1", name="ss1")
                nc.scalar.activation(out=junk[:], in_=st[g]["s"][:],
                                     func=ACTF.Sign, scale=-1.0, bias=t0b[:],
                                     accum_out=ss1[:])
                st[g]["ss1"] = ss1

            def s5_select(g):
                s = st[g]["s"]
                c1 = psm.tile([128, 1], F32, tag="c1", name="c1")
                nc.vector.tensor_scalar(out=c1[:], in0=st[g]["ss1"][:],
                                        scalar1=0.5, scalar2=float(N) / 2.0,
                                        op0=ALU.mult, op1=ALU.add)
                u1 = psm.tile([128, 1], F32, tag="u1", name="u1")
                nc.vector.tensor_scalar(out=u1[:], in0=c1[:],
                                        scalar1=float(A2n), scalar2=float(A1n),
                                        op0=ALU.mult, op1=ALU.add)
                t1 = psm.tile([128, 1], F32, tag="t1", name="t1")
                nc.scalar.activation(out=t1[:], in_=c1[:],
                                     func=ACTF.Copy, scale=u1[:],
                                     bias=float(A0n))
                maskc = pjunk.tile([128, N], F32, tag="maskc", name="maskc")
                nc.vector.tensor_scalar(out=maskc[:], in0=s[:],
                                        scalar1=t1[:],
                                        scalar2=None, op0=ALU.is_le)
                scanc = pmid.tile([128, N], F32, tag="scanc", name="scanc")
                nc.vector.tensor_tensor_scan(
                    out=scanc[:], data0=maskc[:], data1=maskc[:], initial=0.0,
                    op0=ALU.add, op1=ALU.bypass)
                mc = pmid.tile([128, N], F32, tag="mc", name="mc")
                nc.vector.scalar_tensor_tensor(
                    out=mc[:], in0=scanc[:], scalar=float(NCAND), in1=maskc[:],
                    op0=ALU.min, op1=ALU.mult)
                idxs2 = pmid.tile([128, N, 2], I16, tag="idxs2", name="idxs2")
                nc.vector.scalar_tensor_tensor(
                    out=idxs2[:],
                    in0=mc[:].unsqueeze(2).broadcast_to([128, N, 2]),
                    scalar=2.0,
                    in1=parity[:].unsqueeze(1).broadcast_to([128, N, 2]),
                    op0=ALU.mult, op1=ALU.add)
                st[g]["idxs2"] = idxs2

            def s9_scatter(g):
                s, xy16, idxs2 = st[g]["s"], st[g]["xy16"], st[g]["idxs2"]
                scb = ptiny.tile([128, 2 * NCAND], U16, tag="scb", name="scb")
                nc.gpsimd.local_scatter(
                    out_ap=scb[:], data_ap=s[:].bitcast(U16),
                    idxs_ap=idxs2[:].rearrange("p n c -> p (n c)"),
                    channels=128, num_elems=2 * NCAND, num_idxs=2 * N)
                xyc = ptiny.tile([128, 2 * NCAND], F16, tag="xyc", name="xyc",
                                 bufs=3)
                nc.gpsimd.local_scatter(
                    out_ap=xyc[:], data_ap=xy16[:].rearrange("p n c -> p (n c)"),
                    idxs_ap=idxs2[:].rearrange("p n c -> p (n c)"),
                    channels=128, num_elems=2 * NCAND, num_idxs=2 * N)
                st[g]["scb"], st[g]["xyc"] = scb, xyc

            def s10_refine(g):
                sc = st[g]["scb"][:].bitcast(F32)       # [128, NCAND] exact
                w48 = ptiny.tile([128, W], F32, tag="w48", name="w48")
                zc = sc
                for r in range(WIN_ROUNDS):
                    nc.vector.max(out=w48[:, 8 * r:8 * (r + 1)], in_=zc)
                    if r + 1 < WIN_ROUNDS:
                        zn = ptiny.tile([128, NCAND], F32, tag=f"zr{r % 2}",
                                        name="zn")
                        nc.vector.match_replace(
                            out=zn[:], in_to_replace=w48[:, 8 * r:8 * (r + 1)],
                            in_values=zc, imm_value=NEG_BIG)
                        zc = zn[:]
                # rank-50 threshold is always at descending index NCAND-50
                mask2 = ptiny.tile([128, NCAND], F32, tag="mask2", name="mask2")
                nc.vector.tensor_scalar(out=mask2[:],
                                        in0=st[g]["scb"][:].bitcast(F32),
                                        scalar1=w48[:, TIDX:TIDX + 1],
                                        scalar2=None, op0=ALU.is_le)
                scan2 = ptiny.tile([128, NCAND], F32, tag="scan2", name="scan2")
                nc.vector.tensor_tensor_scan(
                    out=scan2[:], data0=mask2[:], data1=mask2[:], initial=0.0,
                    op0=ALU.add, op1=ALU.bypass)
                m2 = ptiny.tile([128, NCAND], F32, tag="m2", name="m2")
                nc.vector.scalar_tensor_tensor(
                    out=m2[:], in0=scan2[:], scalar=64.0, in1=mask2[:],
                    op0=ALU.min, op1=ALU.mult)
                idxs3 = ptiny.tile([128, NCAND, 2], I16, tag="idxs3",
                                   name="idxs3")
                nc.vector.scalar_tensor_tensor(
                    out=idxs3[:],
                    in0=m2[:].unsqueeze(2).broadcast_to([128, NCAND, 2]),
                    scalar=2.0,
                    in1=parity[:].unsqueeze(1).broadcast_to([128, NCAND, 2]),
                    op0=ALU.mult, op1=ALU.add)
                st[g]["idxs3"] = idxs3

            def s11_scatter2(g):
                xyf = ptiny.tile([128, 128], F16, tag="xyf", name="xyf")
                nc.gpsimd.local_scatter(
                    out_ap=xyf[:], data_ap=st[g]["xyc"][:],
                    idxs_ap=st[g]["idxs3"][:].rearrange("p n c -> p (n c)"),
                    channels=128, num_elems=128, num_idxs=2 * NCAND)
                st[g]["xyf"] = xyf

            def s12_dxy(g):
                pcv = st[g]["xyf"][:].rearrange("p (k c) -> p k c", c=2)
                pi = pcv[:, 0:L, :].unsqueeze(2).broadcast_to([128, L, L, 2])
                pj = pcv[:, 0:L, :].unsqueeze(1).broadcast_to([128, L, L, 2])
                dxy = pdist.tile([128, L, L, 2], F16, tag="dxy", name="dxy")
                nc.vector.tensor_tensor(out=dxy[:], in0=pi, in1=pj,
                                        op=ALU.subtract)
                st[g]["dxy"] = dxy

            def s13_sq(g):
                # separate contiguous sqx/sqy via ACT strided-read squares:
                # strided reads are cheap on ACT (~2.4us) but pathological on
                # DVE TT (7.9us measured for the strided pair-add)
                dxy = st[g]["dxy"]
                dv = dxy[:].rearrange("p a b c -> p (a b) c")
                sqx = pdist.tile([128, L * L], F16, tag="sqx", name="sqx")
                sd2x = psm.tile([128, 1], F32, tag="sd2x", name="sd2x")
                nc.scalar.activation(out=sqx[:], in_=dv[:, :, 0],
                                     func=ACTF.Square, accum_out=sd2x[:])
                sqy = pdist.tile([128, L * L], F16, tag="sqy", name="sqy")
                sd2y = psm.tile([128, 1], F32, tag="sd2y", name="sd2y")
                nc.scalar.activation(out=sqy[:], in_=dv[:, :, 1],
                                     func=ACTF.Square, accum_out=sd2y[:])
                st[g]["sqx"], st[g]["sqy"] = sqx, sqy
                st[g]["sd2x"], st[g]["sd2y"] = sd2x, sd2y

            def s14_d2(g):
                d2t = pdist.tile([128, L, L], F16, tag="d2t", name="d2t")
                nc.vector.tensor_tensor(
                    out=d2t[:].rearrange("p a b -> p (a b)"),
                    in0=st[g]["sqx"][:], in1=st[g]["sqy"][:], op=ALU.add)
                st[g]["d2t"] = d2t
                # max(d2) via one f16 2x fold + 1x reduce
                d2v = d2t[:].rearrange("p a b -> p (a b)")
                f1 = pfold.tile([128, 1250], F16, tag="f1", name="f1")
                nc.vector.tensor_tensor(out=f1[:], in0=d2v[:, 0:1250],
                                        in1=d2v[:, 1250:2500], op=ALU.max)
                maxd2 = psm.tile([128, 1], F32, tag="maxd2", name="maxd2")
                nc.vector.tensor_reduce(out=maxd2[:], in_=f1[:],
                                        axis=mybir.AxisListType.X, op=ALU.max)
                st[g]["maxd2"] = maxd2

            def s15_sqrt(g):
                dist = pdist.tile([128, L, L], F16, tag="dist", name="dist")
                sd = psm.tile([128, 1], F32, tag="sd", name="sd")
                nc.scalar.activation(out=dist[:], in_=st[g]["d2t"][:],
                                     func=ACTF.Sqrt, accum_out=sd[:])
                # sum(d^2) = sum(sqx) + sum(sqy): free accums from s13
                sd2 = psm.tile([128, 1], F32, tag="sd2", name="sd2")
                nc.vector.tensor_tensor(out=sd2[:], in0=st[g]["sd2x"][:],
                                        in1=st[g]["sd2y"][:], op=ALU.add)
                q1 = psm.tile([128, 1], F32, tag="q1", name="q1")
                nc.scalar.activation(out=q1[:], in_=sd[:], func=ACTF.Square,
                                     scale=1.0 / 50.0)
                q3 = psm.tile([128, 1], F32, tag="q3", name="q3")
                nc.scalar.activation(out=q3[:], in_=st[g]["maxd2"][:],
                                     func=ACTF.Sqrt,
                                     scale=float(a) * float(a))
                negmur = psm.tile([128, 1], F32, tag="negmur", name="negmur")
                nc.scalar.activation(out=negmur[:], in_=sd[:], func=ACTF.Copy,
                                     scale=-1.0 / float(L))
                st[g].update(dist=dist, sd=sd, sd2=sd2, q1=q1, q3=q3,
                             negmur=negmur)

            def s16_rows(g):
                rows_t = pout.tile([128, L], F32, tag="rows", name="rows_t")
                nc.vector.tensor_reduce(out=rows_t[:], in_=st[g]["dist"][:],
                                        axis=mybir.AxisListType.X, op=ALU.add)
                s2 = psm.tile([128, 1], F32, tag="s2", name="s2")
                nc.vector.tensor_tensor(out=s2[:], in0=st[g]["sd2"][:],
                                        in1=st[g]["q1"][:], op=ALU.subtract)
                osb = pout.tile([128, 6], F32, tag="osb", name="osb")
                nc.vector.memset(osb[:, 2:3], float(b))
                nc.vector.memset(osb[:, 4:5], CONN_CONST)
                st[g].update(rows_t=rows_t, s2=s2, osb=osb)

            def s17_stats(g):
                osb, rows_t = st[g]["osb"], st[g]["rows_t"]
                nc.scalar.activation(out=osb[:, 0:1], in_=st[g]["sd"][:],
                                     func=ACTF.Copy, scale=float(a) / 2500.0,
                                     bias=float(b))
                nc.scalar.activation(out=osb[:, 1:2], in_=st[g]["s2"][:],
                                     func=ACTF.Sqrt,
                                     scale=float(a) * float(a) / 2499.0)
                nc.scalar.activation(out=osb[:, 3:4], in_=st[g]["q3"][:],
                                     func=ACTF.Copy, bias=float(b))
                scr50 = pout.tile([128, L], F32, tag="scr50", name="scr50")
                s2r = psm.tile([128, 1], F32, tag="s2r", name="s2r")
                nc.scalar.activation(out=scr50[:], in_=rows_t[:],
                                     func=ACTF.Square, bias=st[g]["negmur"][:],
                                     accum_out=s2r[:])
                nc.scalar.activation(out=osb[:, 5:6], in_=s2r[:], func=ACTF.Sqrt,
                                     scale=float(a) * float(a) / 49.0)

            def s18_out(g):
                nc.sync.dma_start(out=out_t[g * 128:(g + 1) * 128],
                                  in_=st[g]["osb"][:])
                st[g].clear()

            stages = [s0_dma, s1_xy16, s2_score, s3_count1,
                      s5_select, s9_scatter, s10_refine,
                      s11_scatter2, s12_dxy, s13_sq, s14_d2,
                      s15_sqrt, s16_rows, s17_stats, s18_out]
            S = len(stages)
            for i in range(groups + S - 1):
                for off, fn in enumerate(stages):
                    g = i - off
                    if 0 <= g < groups:
                        fn(g)

    nc.compile()
    return nc


def _const_inputs():
    return {}


def _numpy_fallback(pc, consts):
    """Degenerate-parameter path (sigma==0 or a==0). Exact, CPU."""
    B = pc.shape[0]
    a, b = consts["a"], consts["b"]
    w = np.array([consts["w0"], consts["w1"]], np.float32)
    out = np.zeros((B, 6), np.float32)
    for i in range(B):
        s = pc[i] @ w
        idx = np.argsort(-s, kind="stable")[:L]
        Lp = pc[i, np.sort(idx)]
        d = np.sqrt(((Lp[:, None] - Lp[None, :]) ** 2).sum(-1))
        sc = d * a + b
        fl = np.sort(sc.ravel())
        med = fl[(L * L - 1) // 2]
        out[i] = [sc.mean(), sc.std(ddof=1), fl[0], fl[-1],
                  (sc < med).mean(), sc.sum(1).std(ddof=1)]
    return out


N_LAUNCH = 1


def _get_runner(consts, nb):
    """Build (once) the Bacc program + a jitted 8-core sharded callable."""
    key = (consts["w0"], consts["w1"], consts["a"], consts["b"], nb)
    if key in _CACHE:
        return _CACHE[key]

    import jax
    from jax.sharding import Mesh, PartitionSpec
    from jax.experimental.shard_map import shard_map
    import concourse.bass2jax as b2j
    import concourse.mybir as mybir

    nc = _build_program(consts, nb)
    b2j.install_neuronx_cc_hook()

    in_names, out_names, out_avals, zeros = [], [], [], []
    misc_inputs = {}
    for alloc in nc.m.functions[0].allocations:
        if not isinstance(alloc, mybir.MemoryLocationSet):
            continue
        name = alloc.memorylocations[0].name
        if alloc.kind == "ExternalInput":
            in_names.append(name)
            misc_inputs[name] = (tuple(alloc.tensor_shape),
                                 mybir.dt.np(alloc.dtype))
        elif alloc.kind == "ExternalOutput":
            out_names.append(name)
            shape = tuple(alloc.tensor_shape)
            dtype = mybir.dt.np(alloc.dtype)
            out_avals.append(jax.core.ShapedArray(shape, dtype))
            zeros.append(np.zeros(shape, dtype))
    n_params = len(in_names)
    all_names = in_names + out_names

    def _body(*args):
        return tuple(b2j._bass_exec_p.bind(
            *args, out_avals=tuple(out_avals), in_names=tuple(all_names),
            out_names=tuple(out_names), lowering_input_output_aliases=(),
            sim_require_finite=False, sim_require_nnan=False, nc=nc))

    devices = jax.devices()[:N_CORES]
    mesh = Mesh(np.asarray(devices), ("core",))
    spec = PartitionSpec("core")
    fn = jax.jit(shard_map(
        _body, mesh=mesh, in_specs=(spec,) * (n_params + len(out_names)),
        out_specs=(spec,) * len(out_names), check_rep=False))
    runner = dict(nc=nc, fn=fn, in_names=in_names, out_names=out_names,
                  zeros=zeros, mesh=mesh, spec=spec, misc_inputs=misc_inputs)
    _CACHE[key] = runner
    return runner


def run_shards(runner, pc_shards):
    """Execute one launch: pc_shards is a list of N_CORES per-core arrays."""
    import jax
    from jax.sharding import NamedSharding

    cin = _const_inputs()
    sharding = NamedSharding(runner["mesh"], runner["spec"])
    gather = []
    for name in runner["in_names"]:
        if name == "pc":
            gather.append(np.concatenate(pc_shards, axis=0))
        elif name not in cin:  # partition_id etc. — unused, any value works
            shape, dtype = runner["misc_inputs"][name]
            gather.append(np.zeros((shape[0] * N_CORES,) + tuple(shape[1:]),
                                   dtype))
        else:
            gather.append(np.concatenate([cin[name]] * N_CORES, axis=0))
    gather += [np.concatenate([z] * N_CORES, axis=0) for z in runner["zeros"]]
    args = [jax.device_put(a, sharding) for a in gather]
    outs = runner["fn"](*args)
    outs = [np.asarray(o) for o in outs]
    return dict(zip(runner["out_names"], outs))


def kernel(point_cloud, attn_w, attn_b, filtration_weights, distance_bias):
    pc = np.ascontiguousarray(np.asarray(point_cloud, dtype=np.float32))
    B = pc.shape[0]
    consts = _host_constants(attn_w, filtration_weights, distance_bias)
    if consts["sigma"] == 0.0 or consts["a"] == 0.0:
        return _numpy_fallback(pc, consts)

    nb_core = B // N_CORES           # rows per core overall
    nb = nb_core // N_LAUNCH         # rows per core per launch
    runner = _get_runner(consts, nb)

    outs = []
    for h in range(N_LAUNCH):
        shards = [pc[c * nb_core + h * nb: c * nb_core + (h + 1) * nb]
                  for c in range(N_CORES)]
        res = run_shards(runner, shards)
        outs.append(res["out"].reshape(N_CORES, nb, 6))
    out = np.concatenate(outs, axis=1).reshape(B, 6)
    LAST["dbg"] = None
    LAST["exec_time_ns"] = None
    return out


# ---------------------------------------------------------------------------
# dev: interpreter validation on a small slice (no HW, no neuronxcc)
if __name__ == "__main__":
    import jax
    jax.config.update("jax_default_device", jax.devices("cpu")[0])
    sys.path.insert(0, "/root/problem")
    import reference as ref_mod
    from concourse.bass_interp import MultiCoreSim

    inputs = {k: np.asarray(v) for k, v in ref_mod.setup_inputs().items()}
    consts = _host_constants(inputs["attn_w"], inputs["filtration_weights"],
                             inputs["distance_bias"])
    NB = int(os.environ.get("DEV_NB", "256"))
    pc = np.ascontiguousarray(inputs["point_cloud"][:NB])

    nc = _build_program(consts, NB)
    nc.insert_bir_kernel_barrier_sem_inc()
    sim = MultiCoreSim(nc, 1, require_finite=False, require_nnan=False)
    core = sim.cores[0]
    core.tensor("pc")[:] = pc
    for k, v in _const_inputs().items():
        core.tensor(k)[:] = v
    sim.simulate()
    actual = np.array(core.tensor("out"))
    print("modeled time (ns):", sim.global_time, " per group:",
          sim.global_time / (NB // 128))

    sub = {k: (v[:NB] if k == "point_cloud" else v) for k, v in inputs.items()}
    expected = np.asarray(ref_mod.reference(**{k: np.asarray(v) for k, v in sub.items()}))
    rel = np.abs(actual - expected) / np.maximum(np.abs(expected), 1e-6)
    print("per-col max rel:", np.array2string(rel.max(axis=0), precision=2))
    i = np.unravel_index(rel.argmax(), rel.shape)
    print(f"worst: row {i[0]} col {i[1]} act={actual[i]} exp={expected[i]}")
    print(f"Relative error: {rel.max():.6e}")

